# revision 1
# baseline (speedup 1.0000x reference)
"""DLRM forward (26-table EmbeddingBag + dot interaction + MLPs) on 8 trn2 cores.

Strategy: batch-parallel across the 8 cores (2048 samples each), embedding
tables replicated in each core's HBM so no collectives are needed.

Per-core pipeline (all on device):
  bottom MLP (feature-major, PE) -> xbot [64, BC]
  gather: SWDGE indirect DMA from flat emb table [T*V, 64]; bag-sum pooling
          is done in-DMA via compute_op=add chains (4 gathers per group)
  pooled [samples, t, d] -> PE transposes -> arr [64, (q, s, i)] per 128-sample
          tile, where col block q holds the 4 samples' 27 T-vectors (i=0 is x)
  syrk: gram_q = arr_q^T arr_q -> diagonal 32x32 blocks hold Z per sample
  G2 (diag-block extract, strided DVE copies) -> PE transposes -> G2T with
          layout [27*qq + j, (k, s, i)]  (sample = 4*(4k+qq)+s)
  tril-contraction: Z never extracted per-sample; instead contracted straight
          against top-MLP layer-0 weights grouped by pair row index i
          (lhsT = W_i constant), accumulated in PSUM with the x-part matmul
  top MLP layers 1/2 + sigmoid -> out [4, BC/4] (qq-blocks; host unpermutes)
"""

import numpy as np
from contextlib import ExitStack

import concourse.bass as bass
import concourse.bacc as bacc
import concourse.tile as tile
from concourse import mybir
from concourse.bass_utils import run_bass_kernel_spmd
from concourse.masks import make_identity

F32 = mybir.dt.float32
BF16 = mybir.dt.bfloat16
I32 = mybir.dt.int32
AF = mybir.ActivationFunctionType
ALU = mybir.AluOpType

B = 16384
L = 4
D = 64
T = 26
TT = T + 1  # 27
V = 200000
NCORES = 8
BC = B // NCORES  # 2048 per core


def build_program(bc=BC, v=V, gt=1):
    """Build the single-core SPMD Bass program. bc must be a multiple of 128*gt."""
    ntile = bc // 128           # 128-sample tiles
    ng = ntile // gt            # gather groups (gt tiles each)
    kc = bc // 16               # number of k (q-quads)
    ne = bc // 4                # cols per qq output block
    nb = min(512, bc)           # matmul N chunk
    nn = bc // nb

    nc = bacc.Bacc()

    def din(name, shape, dtype=F32):
        return nc.declare_dram_parameter(name, shape, dtype, isOutput=False)

    x_in = din("x_in", [13, bc])
    emb = din("emb", [T * v, D])
    gidx = din("gidx", [ng * 128, gt * T * L], I32)
    wb0t = din("wb0t", [13, 512])
    bb0 = din("bb0", [128, 4])
    wb1t = din("wb1t", [128, 4 * 256])
    bb1 = din("bb1", [128, 2])
    wb2t = din("wb2t", [128, 2 * 64])
    bb2 = din("bb2", [64, 1])
    wt0x = din("wt0x", [64, 512])
    wall = din("wall", [128, T * 512], BF16)
    bt0 = din("bt0", [128, 4])
    wt1t = din("wt1t", [128, 4 * 256])
    bt1 = din("bt1", [128, 2])
    wt2t = din("wt2t", [128, 2])
    bt2 = din("bt2", [1, 1])
    out = nc.declare_dram_parameter("out", [4, ne], F32, isOutput=True)

    with TileCtx(nc) as tc, ExitStack() as ctx:
        cpool = ctx.enter_context(tc.tile_pool(name="const", bufs=1))
        psA = ctx.enter_context(tc.tile_pool(name="psA", bufs=3, space="PSUM"))
        psB = ctx.enter_context(tc.tile_pool(name="psB", bufs=4, space="PSUM"))
        psC = ctx.enter_context(tc.tile_pool(name="psC", bufs=1, space="PSUM"))
        work = ctx.enter_context(tc.tile_pool(name="work", bufs=1))
        pooled_p = ctx.enter_context(tc.tile_pool(name="pooled", bufs=2))
        idx_p = ctx.enter_context(tc.tile_pool(name="idx", bufs=4))
        arr_p = ctx.enter_context(tc.tile_pool(name="arr", bufs=2))
        gram_p = ctx.enter_context(tc.tile_pool(name="gram", bufs=2))
        g2_p = ctx.enter_context(tc.tile_pool(name="g2", bufs=2))
        act_p = ctx.enter_context(tc.tile_pool(name="act", bufs=4))

        def load(dram, shape, dtype=F32):
            t = cpool.tile(shape, dtype, tag=f"c_{dram.name}")
            nc.sync.dma_start(out=t[:], in_=dram[:])
            return t

        ident = cpool.tile([128, 128], F32)
        make_identity(nc, ident[:])
        wb0t_t = load(wb0t, [13, 512])
        bb0_t = load(bb0, [128, 4])
        wb1t_t = load(wb1t, [128, 4 * 256])
        bb1_t = load(bb1, [128, 2])
        wb2t_t = load(wb2t, [128, 2 * 64])
        bb2_t = load(bb2, [64, 1])
        wt0x_t = cpool.tile([128, 512], F32, tag="c_wt0x")
        nc.sync.dma_start(out=wt0x_t[0:64, :], in_=wt0x[:])
        wall_t = load(wall, [128, T * 512], BF16)
        bt0_t = load(bt0, [128, 4])
        wt1t_t = load(wt1t, [128, 4 * 256])
        bt1_t = load(bt1, [128, 2])
        wt2t_t = load(wt2t, [128, 2])
        bt2_t = load(bt2, [1, 1])

        # ---------------- bottom MLP (feature-major, column-blocked) ----------
        xbot_t = work.tile([128, bc], F32)
        xbot = xbot_t[0:64, :]
        xin_t = work.tile([13, bc], F32)
        nc.sync.dma_start(out=xin_t[:], in_=x_in[:])
        with tc.tile_pool(name="bot", bufs=2) as bot_p:
            for n in range(nn):
                y0n = bot_p.tile([128, 4 * nb], F32, tag="y0n")
                for m in range(4):
                    ps = psB.tile([128, nb], F32)
                    nc.tensor.matmul(
                        ps[:],
                        lhsT=wb0t_t[:, m * 128:(m + 1) * 128],
                        rhs=xin_t[:, n * nb:(n + 1) * nb],
                        start=True, stop=True,
                    )
                    nc.scalar.activation(
                        y0n[:, m * nb:(m + 1) * nb],
                        ps[:], AF.Relu, bias=bb0_t[:, m:m + 1],
                    )
                y1n = bot_p.tile([128, 2 * nb], F32, tag="y1n")
                for m in range(2):
                    ps = psB.tile([128, nb], F32)
                    for k in range(4):
                        nc.tensor.matmul(
                            ps[:],
                            lhsT=wb1t_t[:, k * 256 + m * 128: k * 256 + (m + 1) * 128],
                            rhs=y0n[:, k * nb:(k + 1) * nb],
                            start=(k == 0), stop=(k == 3),
                        )
                    nc.scalar.activation(
                        y1n[:, m * nb:(m + 1) * nb],
                        ps[:], AF.Relu, bias=bb1_t[:, m:m + 1],
                    )
                ps = psB.tile([128, nb], F32)
                for k in range(2):
                    nc.tensor.matmul(
                        ps[:64, :],
                        lhsT=wb2t_t[:, k * 64:(k + 1) * 64],
                        rhs=y1n[:, k * nb:(k + 1) * nb],
                        start=(k == 0), stop=(k == 1),
                    )
                nc.scalar.activation(
                    xbot[:, n * nb:(n + 1) * nb], ps[:64, :], AF.Relu,
                    bias=bb2_t[:, 0:1],
                )
        xbv = xbot.rearrange("d (k r s) -> d k r s", r=4, s=4)

        # ---------------- gather + pool + interaction prep ----------------
        # two G2T tiles: A holds qq 0 (rows 0..26) and qq 1 (rows 32..58),
        # B holds qq 2 / qq 3 likewise -- matmul bases stay in {0, 32}
        g2ta = work.tile([64, kc * 128], BF16)
        g2tb = work.tile([64, kc * 128], BF16)
        for g in range(ng):
            pooled = pooled_p.tile([128, gt * T * D], F32)
            pview = pooled[:].rearrange("p (c d) -> p c d", d=D)
            it = idx_p.tile([128, gt * T * L], I32)
            nc.sync.dma_start(
                out=it[:], in_=gidx[g * 128:(g + 1) * 128, :]
            )
            # HW indirect DMA supports exactly one offset per dest partition,
            # so each (table, bag-elem) is its own 128-row gather; the bag sum
            # is accumulated in-DMA via the CCE add op.
            for c in range(gt * T):
                for l in range(4):
                    nc.gpsimd.indirect_dma_start(
                        out=pview[:, c, :],
                        out_offset=None,
                        in_=emb[:],
                        in_offset=bass.IndirectOffsetOnAxis(
                            ap=it[:, c * L + l:c * L + l + 1], axis=0),
                        compute_op=(ALU.bypass if l == 0 else ALU.add),
                    )
            for u in range(gt):
                tg = g * gt + u  # global tile == arr chunk
                arr_c = arr_p.tile([64, 32 * 128], BF16)
                arr_v = arr_c[:].rearrange("d (q s i) -> d q s i", s=4, i=32)
                # zero the pad cols i=27..31 (syrk reads full 128-col blocks)
                nc.vector.memset(arr_v[:, :, :, 27:32], 0.0)
                for up in range(13):
                    pst = psA.tile([128, 128], F32, tag="t128")
                    nc.tensor.transpose(
                        pst[:],
                        pooled[:, (u * T + 2 * up) * D:(u * T + 2 * up + 2) * D],
                        ident[:],
                    )
                    for h in range(2):
                        nc.any.tensor_copy(
                            out=arr_v[:, :, :, 1 + 2 * up + h],
                            in_=pst[h * 64:(h + 1) * 64, :].rearrange(
                                "d (q s) -> d q s", s=4),
                        )
                nc.vector.tensor_copy(
                    out=arr_v[:, :, :, 0],
                    in_=xbot_t[0:64, tg * 128:(tg + 1) * 128].rearrange(
                        "d (q s) -> d q s", s=4),
                )
                g2_c = g2_p.tile([128, 32 * 32], F32)
                nc.vector.memset(g2_c[:], 0.0)
                for half in range(4):
                    gram_c = gram_p.tile([128, 8 * 128], F32)
                    for ql in range(8):
                        q_loc = half * 8 + ql
                        psg = psA.tile([128, 128], F32, tag="t128")
                        nc.tensor.matmul(
                            psg[:],
                            lhsT=arr_c[:, q_loc * 128:(q_loc + 1) * 128],
                            rhs=arr_c[:, q_loc * 128:(q_loc + 1) * 128],
                            start=True, stop=True,
                        )
                        nc.any.tensor_copy(
                            out=gram_c[:, ql * 128:(ql + 1) * 128], in_=psg[:]
                        )
                    for s in range(4):
                        nc.vector.tensor_copy(
                            out=g2_c[32 * s:32 * s + 27, :].rearrange(
                                "i (q j) -> i q j", j=32)[
                                :, half * 8:(half + 1) * 8, 0:27],
                            in_=gram_c[32 * s:32 * s + 27, :].rearrange(
                                "i (q c) -> i q c", c=128)[:, :, 32 * s:32 * s + 27],
                        )
                for w in range(8):
                    k = tg * 8 + w
                    psta = psA.tile([64, 128], F32, tag="t128")
                    nc.tensor.transpose(
                        psta[:], g2_c[:, w * 128:w * 128 + 64], ident[:]
                    )
                    nc.any.tensor_copy(
                        out=g2ta[:, k * 128:(k + 1) * 128], in_=psta[:]
                    )
                    pstb = psA.tile([64, 128], F32, tag="t128")
                    nc.tensor.transpose(
                        pstb[:], g2_c[:, w * 128 + 64:(w + 1) * 128], ident[:]
                    )
                    nc.any.tensor_copy(
                        out=g2tb[:, k * 128:(k + 1) * 128], in_=pstb[:]
                    )

        # ---------------- tril-contraction + top MLP ----------------
        for qq in range(4):
            g2half = g2ta if qq < 2 else g2tb
            hb = 32 * (qq % 2)
            g2s = g2half[hb:hb + 27, :].rearrange(
                "j (k s i) -> j k s i", s=4, i=32)
            y0q = []
            for m in range(4):
                ps = psB.tile([128, ne], F32)
                for i in range(1, TT):
                    nc.tensor.matmul(
                        ps[:],
                        lhsT=wall_t[hb:hb + 27,
                                    (i - 1) * 512 + m * 128:
                                    (i - 1) * 512 + (m + 1) * 128],
                        rhs=g2s[:, :, :, i],
                        start=(i == 1), stop=False,
                    )
                nc.tensor.matmul(
                    ps[:],
                    lhsT=wt0x_t[0:64, m * 128:(m + 1) * 128],
                    rhs=xbv[:, :, qq, :],
                    start=False, stop=True,
                )
                t0 = act_p.tile([128, ne], F32, tag="yq")
                nc.scalar.activation(t0[:], ps[:], AF.Relu, bias=bt0_t[:, m:m + 1])
                y0q.append(t0)
            y1q = []
            for m in range(2):
                ps = psB.tile([128, ne], F32)
                for k in range(4):
                    nc.tensor.matmul(
                        ps[:],
                        lhsT=wt1t_t[:, k * 256 + m * 128: k * 256 + (m + 1) * 128],
                        rhs=y0q[k][:],
                        start=(k == 0), stop=(k == 3),
                    )
                t1 = act_p.tile([128, ne], F32, tag="yq")
                nc.scalar.activation(t1[:], ps[:], AF.Relu, bias=bt1_t[:, m:m + 1])
                y1q.append(t1)
            ps2 = psC.tile([1, ne], F32)
            for k in range(2):
                nc.tensor.matmul(
                    ps2[:],
                    lhsT=wt2t_t[:, k:k + 1],
                    rhs=y1q[k][:],
                    start=(k == 0), stop=(k == 1),
                )
            ot = act_p.tile([1, ne], F32, tag="yq")
            nc.scalar.activation(ot[:], ps2[:], AF.Sigmoid, bias=bt2_t[:, 0:1])
            nc.sync.dma_start(out=out[qq:qq + 1, :], in_=ot[:])

    nc.finalize()
    return nc


def TileCtx(nc):
    return tile.TileContext(nc)


# ---------------------------------------------------------------------------
# host-side packing
# ---------------------------------------------------------------------------

def pack_weights(ws):
    """ws: dict of reference weight arrays -> dict of packed f32 arrays."""
    f = lambda a: np.ascontiguousarray(a, dtype=np.float32)
    o = {}
    o["wb0t"] = f(ws["Wb0"].T)                         # [13, 512]
    o["bb0"] = f(ws["bb0"].reshape(4, 128).T)          # [128, 4]
    w1 = ws["Wb1"].T                                   # [512, 256]
    o["wb1t"] = f(np.concatenate([w1[128 * k:128 * (k + 1)] for k in range(4)], 1))
    o["bb1"] = f(ws["bb1"].reshape(2, 128).T)
    w2 = ws["Wb2"].T                                   # [256, 64]
    o["wb2t"] = f(np.concatenate([w2[128 * k:128 * (k + 1)] for k in range(2)], 1))
    o["bb2"] = f(ws["bb2"].reshape(64, 1))
    wt0 = np.asarray(ws["Wt0"], dtype=np.float64)      # [512, 415]
    o["wt0x"] = f(wt0[:, :64].T)                       # [64, 512]
    wall = np.zeros((128, T * 512), dtype=np.float32)
    for i in range(1, TT):
        off = i * (i - 1) // 2
        # W_i[j, m] = Wt0[m, 64 + off + j] for j < i; replicated in all
        # four 32-row bands so lhsT base always matches rhs base 32*qq
        for qq in range(4):
            wall[32 * qq:32 * qq + i, (i - 1) * 512:i * 512] = \
                wt0[:, 64 + off:64 + off + i].T
    import ml_dtypes
    o["wall"] = wall.astype(ml_dtypes.bfloat16)
    o["bt0"] = f(ws["bt0"].reshape(4, 128).T)
    t1 = ws["Wt1"].T                                   # [512, 256]
    o["wt1t"] = f(np.concatenate([t1[128 * k:128 * (k + 1)] for k in range(4)], 1))
    o["bt1"] = f(ws["bt1"].reshape(2, 128).T)
    t2 = ws["Wt2"].T                                   # [256, 1]
    o["wt2t"] = f(np.concatenate([t2[128 * k:128 * (k + 1)] for k in range(2)], 1))
    o["bt2"] = f(ws["bt2"].reshape(1, 1))
    return o


def pack_gidx(lsi_core, bc, v, gt=1):
    """lsi_core: [T, bc*L] int indices for this core's samples.

    returns [ng*128, gt*T*L] int32 with entry [g*128+p, (u*T+t)*L + l]
      = t*v + lsi_core[t, (128*(g*gt+u)+p)*L + l]
    """
    ntile = bc // 128
    ng = ntile // gt
    li = np.asarray(lsi_core).reshape(T, bc, L)        # [t, n, l]
    li = li.reshape(T, ng, gt, 128, L)                 # [t, g, u, p, l]
    gi = li + (np.arange(T, dtype=np.int64) * v)[:, None, None, None, None]
    gi = gi.transpose(1, 3, 2, 0, 4)                   # [g, p, u, t, l]
    return np.ascontiguousarray(
        gi.reshape(ng * 128, gt * T * L), dtype=np.int32)


def unpermute_out(out_c, bc):
    """out_c: [4, bc/4] (qq, 4k+s) -> [bc] in natural sample order l'=16k+4qq+s."""
    kc = bc // 16
    o = out_c.reshape(4, kc, 4)        # [qq, k, s]
    o = o.transpose(1, 0, 2)           # [k, qq, s]
    return np.ascontiguousarray(o.reshape(bc))


_PROG = None
TRACE = False          # set by test harness to collect an NTFF profile
TRACE_KW = {}
LAST_RESULTS = None    # BassKernelResults of the most recent run


def kernel(**inputs):
    global _PROG, LAST_RESULTS
    dense_x = np.asarray(inputs["dense_x"], dtype=np.float32)     # [B, 13]
    lsi = np.asarray(inputs["lS_i"]).reshape(T, B * L)
    emb = np.asarray(inputs["emb"], dtype=np.float32)             # [T, V, D]
    emb_flat = np.ascontiguousarray(emb.reshape(T * V, D))

    ws = {k: np.asarray(inputs[k]) for k in (
        "Wb0", "bb0", "Wb1", "bb1", "Wb2", "bb2",
        "Wt0", "bt0", "Wt1", "bt1", "Wt2", "bt2")}
    packed_w = pack_weights(ws)

    if _PROG is None:
        _PROG = build_program()
    nc = _PROG

    lsi_r = lsi.reshape(T, B, L)
    in_maps = []
    for c in range(NCORES):
        sl = slice(BC * c, BC * (c + 1))
        m = dict(packed_w)
        m["x_in"] = np.ascontiguousarray(dense_x[sl].T)           # [13, BC]
        m["gidx"] = pack_gidx(lsi_r[:, sl, :].reshape(T, BC * L), BC, V)
        m["emb"] = emb_flat
        in_maps.append(m)

    bkr = run_bass_kernel_spmd(
        nc, in_maps, list(range(NCORES)), trace=TRACE, **TRACE_KW)
    LAST_RESULTS = bkr
    outs = [unpermute_out(np.asarray(r["out"]), BC) for r in bkr.results]
    return np.concatenate(outs).reshape(B, 1).astype(np.float32)



# revision 9
# speedup vs baseline: 1.0996x; 1.0996x over previous
"""DLRM forward (26-table EmbeddingBag + dot interaction + MLPs) on 8 trn2 cores.

Batch-parallel across cores (2048 samples each), tables replicated.

Per-core pipeline:
  bottom MLP (feature-major, bf16 weights/acts, PE) -> xbot [64, BC] bf16
  gather (mode-dependent) -> pooled vectors per (sample, table)
  per 128-sample tile: PE transposes pooled -> arr [64, (q, s, i)] bf16
  syrk per q-block: gram = arr_q^T arr_q; per-sample Z in diagonal 32x32 blocks
  symmetric extract: Z[i,j] = Z[j,i], so j-major rows are read straight from
  the gram columns (no second transpose); DVE copies PSUM -> g2t bf16
  top MLP layer0 = 26 z-contraction matmuls (lhsT = W_i, rows j<i) + x-part,
  layers 1/2 + sigmoid -> out [1, BC] in natural sample order
"""

import numpy as np
from contextlib import ExitStack

import concourse.bass as bass
import concourse.bacc as bacc
import concourse.tile as tile
from concourse import mybir
from concourse.bass_utils import run_bass_kernel_spmd
from concourse.masks import make_identity

F32 = mybir.dt.float32
BF16 = mybir.dt.bfloat16
I16 = mybir.dt.int16
I32 = mybir.dt.int32
AF = mybir.ActivationFunctionType
ALU = mybir.AluOpType

B = 16384
L = 4
D = 64
T = 26
TT = T + 1
V = 200000
NCORES = 8
BC = B // NCORES          # 2048
NTILE = BC // 128         # 16
NGRP = NTILE // 4         # 4 groups of 4 tiles (512 samples)

GATHER_MODE = "chain"     # "chain" | "bounce"
W16 = 32768               # int16 gather window
NW = (V + W16 - 1) // W16  # 7


def build_program(caps=None):
    """caps: per-window gather capacities (sbufgather mode), else None."""
    nc = bacc.Bacc()

    def din(name, shape, dtype=F32):
        return nc.declare_dram_parameter(name, shape, dtype, isOutput=False)

    emb = din("emb", [T * V, D])
    x_in = din("x_in", [13, BC], BF16)
    wb0t = din("wb0t", [13, 512], BF16)
    bb0 = din("bb0", [128, 4])
    wb1t = din("wb1t", [128, 4 * 256], BF16)
    bb1 = din("bb1", [128, 2])
    wb2t = din("wb2t", [128, 2 * 64], BF16)
    bb2 = din("bb2", [64, 1])
    wt0x = din("wt0x", [64, 512], BF16)
    wall = din("wall", [32, T * 512], BF16)
    bt0 = din("bt0", [128, 4])
    wt1t = din("wt1t", [128, 4 * 256], BF16)
    bt1 = din("bt1", [128, 2])
    wt2t = din("wt2t", [128, 2], BF16)
    bt2 = din("bt2", [1, 1])
    out = nc.declare_dram_parameter("out", [1, BC], F32, isOutput=True)

    if GATHER_MODE == "chain":
        gidx = din("gidx", [NTILE * 128, L * T], I32)
    else:
        CT = sum(caps)                      # stream rows per table
        CC = CT // 128                      # stream chunks per table
        assert CT % 128 == 0 and CT <= 32767
        g1x = din("g1x", [128, T * CT // 16], I16)
        g2x = din("g2x", [128, T * 512], I16)

    with tile.TileContext(nc) as tc, ExitStack() as ctx:
        cpool = ctx.enter_context(tc.tile_pool(name="const", bufs=1))
        psT = ctx.enter_context(tc.tile_pool(name="psT", bufs=2, space="PSUM"))
        psG = ctx.enter_context(tc.tile_pool(name="psG", bufs=1, space="PSUM"))
        psB = ctx.enter_context(tc.tile_pool(name="psB", bufs=2, space="PSUM"))
        psC = ctx.enter_context(tc.tile_pool(name="psC", bufs=1, space="PSUM"))
        arr_p = ctx.enter_context(tc.tile_pool(name="arr", bufs=2))
        g2t_p = ctx.enter_context(tc.tile_pool(name="g2t", bufs=2))
        y0_p = ctx.enter_context(tc.tile_pool(name="y0", bufs=4))
        y1_p = ctx.enter_context(tc.tile_pool(name="y1", bufs=2))
        ot_p = ctx.enter_context(tc.tile_pool(name="ot", bufs=2))

        def load(dram, shape, dtype=F32):
            t = cpool.tile(shape, dtype, tag=f"c_{dram.name}")
            nc.sync.dma_start(out=t[:], in_=dram[:])
            return t

        identb = cpool.tile([128, 128], BF16)
        make_identity(nc, identb[:])
        wb0t_t = load(wb0t, [13, 512], BF16)
        bb0_t = load(bb0, [128, 4])
        wb1t_t = load(wb1t, [128, 4 * 256], BF16)
        bb1_t = load(bb1, [128, 2])
        wb2t_t = load(wb2t, [128, 2 * 64], BF16)
        bb2_t = load(bb2, [64, 1])
        wt0x_t = load(wt0x, [64, 512], BF16)
        wall_t = load(wall, [32, T * 512], BF16)
        bt0_t = load(bt0, [128, 4])
        wt1t_t = load(wt1t, [128, 4 * 256], BF16)
        bt1_t = load(bt1, [128, 2])
        wt2t_t = load(wt2t, [128, 2], BF16)
        bt2_t = load(bt2, [1, 1])

        # ---------------- bottom MLP (bf16, feature-major) ----------
        xbot = cpool.tile([64, BC], BF16, tag="xbot")
        xin_t = cpool.tile([13, BC], BF16, tag="xin")
        nc.sync.dma_start(out=xin_t[:], in_=x_in[:])
        nb = 512
        with tc.tile_pool(name="bot", bufs=2) as bot_p:
            for n in range(BC // nb):
                y0n = bot_p.tile([128, 4 * nb], BF16, tag="y0n")
                for m in range(4):
                    ps = psB.tile([128, nb], F32)
                    nc.tensor.matmul(
                        ps[:], lhsT=wb0t_t[:, m * 128:(m + 1) * 128],
                        rhs=xin_t[:, n * nb:(n + 1) * nb],
                        start=True, stop=True)
                    nc.scalar.activation(
                        y0n[:, m * nb:(m + 1) * nb], ps[:], AF.Relu,
                        bias=bb0_t[:, m:m + 1])
                y1n = bot_p.tile([128, 2 * nb], BF16, tag="y1n")
                for m in range(2):
                    ps = psB.tile([128, nb], F32)
                    for k in range(4):
                        nc.tensor.matmul(
                            ps[:],
                            lhsT=wb1t_t[:, k * 256 + m * 128:k * 256 + (m + 1) * 128],
                            rhs=y0n[:, k * nb:(k + 1) * nb],
                            start=(k == 0), stop=(k == 3))
                    nc.scalar.activation(
                        y1n[:, m * nb:(m + 1) * nb], ps[:], AF.Relu,
                        bias=bb1_t[:, m:m + 1])
                ps = psB.tile([128, nb], F32)
                for k in range(2):
                    nc.tensor.matmul(
                        ps[0:64, :], lhsT=wb2t_t[:, k * 64:(k + 1) * 64],
                        rhs=y1n[:, k * nb:(k + 1) * nb],
                        start=(k == 0), stop=(k == 1))
                nc.scalar.activation(
                    xbot[:, n * nb:(n + 1) * nb], ps[0:64, :], AF.Relu,
                    bias=bb2_t[:, 0:1])

        # ---------------- gather producers ----------------
        if GATHER_MODE == "chain":
            pooled_p = ctx.enter_context(tc.tile_pool(name="pooled", bufs=2))
            idx_p = ctx.enter_context(tc.tile_pool(name="idx", bufs=2))
            identf = cpool.tile([128, 128], F32, tag="identf")
            make_identity(nc, identf[:])

            def produce_pooled(u):
                """-> chunk(up) AP factory for tile u's pooled vectors."""
                pooled = pooled_p.tile([128, T * D], F32)
                pv = pooled[:].rearrange("p (c d) -> p c d", d=D)
                it = idx_p.tile([128, L * T], I32)
                nc.sync.dma_start(out=it[:], in_=gidx[u * 128:(u + 1) * 128, :])
                for l in range(L):
                    for t in range(T):
                        nc.gpsimd.indirect_dma_start(
                            out=pv[:, t, :],
                            out_offset=None,
                            in_=emb[:],
                            in_offset=bass.IndirectOffsetOnAxis(
                                ap=it[:, l * T + t:l * T + t + 1], axis=0),
                            compute_op=(ALU.bypass if l == 0 else ALU.add),
                        )
                return (lambda up: pooled[:, 2 * up * D:(2 * up + 2) * D]), identf
        elif GATHER_MODE == "bounce":
            dram_p = ctx.enter_context(tc.tile_pool(name="dramb", bufs=1, space="DRAM"))
            bounce = dram_p.tile([T * CT, D], F32)
            pooledS = cpool.tile([128, NTILE * T * D], BF16, tag="pooledS")
            psv = pooledS[:].rearrange("p (u t d) -> p u t d", u=NTILE, t=T)
            nct = CT // 16
            with tc.tile_pool(name="stream", bufs=2) as stream_p, \
                 tc.tile_pool(name="dstg", bufs=2) as dst_p, \
                 tc.tile_pool(name="g1i", bufs=2) as g1i_p, \
                 tc.tile_pool(name="g2i", bufs=2) as g2i_p:
                for t in range(T):
                    it1 = g1i_p.tile([128, nct], I16)
                    nc.sync.dma_start(out=it1[:], in_=g1x[:, t * nct:(t + 1) * nct])
                    stream = stream_p.tile([128, CC * D], F32)
                    sv = stream[:].rearrange("p (c d) -> p c d", d=D)
                    off = 0
                    for w in range(NW):
                        cw = caps[w]
                        wlen = min(W16, V - w * W16)
                        nc.gpsimd.dma_gather(
                            out_ap=sv[:, off // 128:(off + cw) // 128, :],
                            in_ap=emb[t * V + w * W16:t * V + w * W16 + wlen, :],
                            idxs_ap=it1[:, off // 16:(off + cw) // 16],
                            num_idxs=cw, num_idxs_reg=cw, elem_size=D,
                            single_packet=False,
                        )
                        off += cw
                    bt = bounce[t * CT:(t + 1) * CT, :].rearrange(
                        "(p c) d -> p c d", p=128)
                    nc.sync.dma_start(out=bt, in_=sv[:, :, :])
                    it2 = g2i_p.tile([128, 512], I16)
                    nc.sync.dma_start(out=it2[:], in_=g2x[:, t * 512:(t + 1) * 512])
                    dst = dst_p.tile([128, L * NTILE * D], F32)
                    dv = dst[:].rearrange("p (l u d) -> p l u d", l=L, u=NTILE)
                    nc.gpsimd.dma_gather(
                        out_ap=dst[:].rearrange("p (c d) -> p c d", d=D),
                        in_ap=bounce[t * CT:(t + 1) * CT, :],
                        idxs_ap=it2[:],
                        num_idxs=L * BC, num_idxs_reg=L * BC, elem_size=D,
                        single_packet=False,
                    )
                    nc.vector.tensor_add(out=dv[:, 0], in0=dv[:, 0], in1=dv[:, 1])
                    nc.vector.tensor_add(out=dv[:, 2], in0=dv[:, 2], in1=dv[:, 3])
                    nc.vector.tensor_add(out=psv[:, :, t, :], in0=dv[:, 0],
                                         in1=dv[:, 2])

            def produce_pooled(u):
                return (lambda up: psv[:, u, 2 * up:2 * up + 2, :]), identb
        else:
            raise NotImplementedError(GATHER_MODE)

        # ---------------- interaction + top MLP ----------------
        for g in range(NGRP):
            g2t = g2t_p.tile([32, 4 * 32 * 4 * 32], BF16)
            g2tv = g2t[:].rearrange("j (K s i) -> j K s i", s=4, i=32)
            for ug in range(4):
                u = g * 4 + ug
                chunk, ident = produce_pooled(u)
                arr = arr_p.tile([64, 32 * 128], BF16)
                arr_v = arr[:].rearrange("d (q s i) -> d q s i", s=4, i=32)
                nc.vector.tensor_copy(
                    out=arr_v[:, :, :, 0],
                    in_=xbot[:, u * 128:(u + 1) * 128].rearrange(
                        "d (q s) -> d q s", s=4))
                for up in range(13):
                    pst = psT.tile([128, 128], F32)
                    nc.tensor.transpose(pst[:], chunk(up), ident[:])
                    for h in range(2):
                        nc.any.tensor_copy(
                            out=arr_v[:, :, :, 1 + 2 * up + h],
                            in_=pst[h * 64:(h + 1) * 64, :].rearrange(
                                "d (q s) -> d q s", s=4))
                for half in range(4):
                    psg = psG.tile([128, 8 * 128], F32)
                    for ql in range(8):
                        q = half * 8 + ql
                        nc.tensor.matmul(
                            psg[:, ql * 128:(ql + 1) * 128],
                            lhsT=arr[:, q * 128:(q + 1) * 128],
                            rhs=arr[:, q * 128:(q + 1) * 128],
                            start=True, stop=True)
                    pgv = psg[:].rearrange("p (q c) -> p q c", c=128)
                    for s in range(4):
                        nc.any.tensor_copy(
                            out=g2tv[0:27, ug * 32 + half * 8:ug * 32 + half * 8 + 8,
                                     s, :],
                            in_=pgv[32 * s:32 * s + 27, :, 32 * s:32 * s + 32])

            # ---- top MLP for this group's 512 samples ----
            y0 = []
            for m in range(4):
                ps = psB.tile([128, 512], F32)
                for i in range(1, TT):
                    nc.tensor.matmul(
                        ps[:],
                        lhsT=wall_t[0:27, (i - 1) * 512 + m * 128:
                                    (i - 1) * 512 + (m + 1) * 128],
                        rhs=g2tv[0:27, :, :, i],
                        start=(i == 1), stop=False)
                nc.tensor.matmul(
                    ps[:], lhsT=wt0x_t[:, m * 128:(m + 1) * 128],
                    rhs=xbot[:, g * 512:(g + 1) * 512],
                    start=False, stop=True)
                t0 = y0_p.tile([128, 512], BF16, tag="y0t")
                nc.scalar.activation(t0[:], ps[:], AF.Relu, bias=bt0_t[:, m:m + 1])
                y0.append(t0)
            y1 = []
            for m in range(2):
                ps = psB.tile([128, 512], F32)
                for k in range(4):
                    nc.tensor.matmul(
                        ps[:],
                        lhsT=wt1t_t[:, k * 256 + m * 128:k * 256 + (m + 1) * 128],
                        rhs=y0[k][:],
                        start=(k == 0), stop=(k == 3))
                t1 = y1_p.tile([128, 512], BF16, tag="y1t")
                nc.scalar.activation(t1[:], ps[:], AF.Relu, bias=bt1_t[:, m:m + 1])
                y1.append(t1)
            ps2 = psC.tile([1, 512], F32)
            for k in range(2):
                nc.tensor.matmul(
                    ps2[:], lhsT=wt2t_t[:, k:k + 1], rhs=y1[k][:],
                    start=(k == 0), stop=(k == 1))
            ot = ot_p.tile([1, 512], F32, tag="ott")
            nc.scalar.activation(ot[:], ps2[:], AF.Sigmoid, bias=bt2_t[:, 0:1])
            nc.sync.dma_start(out=out[0:1, g * 512:(g + 1) * 512], in_=ot[:])

    nc.finalize()
    return nc


# ---------------------------------------------------------------------------
# host-side packing
# ---------------------------------------------------------------------------

def _bf16(a):
    import ml_dtypes
    return np.ascontiguousarray(np.asarray(a, dtype=np.float32)).astype(
        ml_dtypes.bfloat16)


def pack_weights(ws):
    f = lambda a: np.ascontiguousarray(np.asarray(a), dtype=np.float32)
    o = {}
    o["wb0t"] = _bf16(np.asarray(ws["Wb0"]).T)                 # [13, 512]
    o["bb0"] = f(np.asarray(ws["bb0"]).reshape(4, 128).T)
    w1 = np.asarray(ws["Wb1"]).T                               # [512, 256]
    o["wb1t"] = _bf16(np.concatenate([w1[128 * k:128 * (k + 1)] for k in range(4)], 1))
    o["bb1"] = f(np.asarray(ws["bb1"]).reshape(2, 128).T)
    w2 = np.asarray(ws["Wb2"]).T                               # [256, 64]
    o["wb2t"] = _bf16(np.concatenate([w2[128 * k:128 * (k + 1)] for k in range(2)], 1))
    o["bb2"] = f(np.asarray(ws["bb2"]).reshape(64, 1))
    wt0 = np.asarray(ws["Wt0"], dtype=np.float64)              # [512, 415]
    o["wt0x"] = _bf16(wt0[:, :64].T)                           # [64, 512]
    wallm = np.zeros((32, T * 512), dtype=np.float32)
    for i in range(1, TT):
        off = i * (i - 1) // 2
        wallm[0:i, (i - 1) * 512:i * 512] = wt0[:, 64 + off:64 + off + i].T
    o["wall"] = _bf16(wallm)
    o["bt0"] = f(np.asarray(ws["bt0"]).reshape(4, 128).T)
    t1 = np.asarray(ws["Wt1"]).T                               # [512, 256]
    o["wt1t"] = _bf16(np.concatenate([t1[128 * k:128 * (k + 1)] for k in range(4)], 1))
    o["bt1"] = f(np.asarray(ws["bt1"]).reshape(2, 128).T)
    t2 = np.asarray(ws["Wt2"]).T                               # [256, 1]
    o["wt2t"] = _bf16(np.concatenate([t2[128 * k:128 * (k + 1)] for k in range(2)], 1))
    o["bt2"] = f(np.asarray(ws["bt2"]).reshape(1, 1))
    return o


def pack_gidx_chain(lsi_core):
    """lsi_core [T, BC, L] -> [NTILE*128, L*T] int32 flat indices (l, t cols)."""
    li = np.asarray(lsi_core).reshape(T, NTILE, 128, L)
    gi = li + (np.arange(T, dtype=np.int64) * V)[:, None, None, None]
    gi = gi.transpose(1, 2, 3, 0)                  # [u, p, l, t]
    return np.ascontiguousarray(gi.reshape(NTILE * 128, L * T), dtype=np.int32)


def _wrap16(lst):
    """[n] int16 -> [128, n//16] wrapped in 16 partitions, replicated 8x."""
    n = len(lst)
    band = np.asarray(lst, dtype=np.int16).reshape(n // 16, 16).T
    return np.tile(band, (8, 1))


def gather_caps(lsi):
    """lsi [T, B, L] -> per-window stream capacities (shared across cores)."""
    caps = []
    w_of = np.asarray(lsi) // W16
    for w in range(NW):
        mx = 0
        for c in range(NCORES):
            m = (w_of[:, BC * c:BC * (c + 1), :] == w).sum(axis=(1, 2)).max()
            mx = max(mx, int(m))
        caps.append(max(128, -(-mx // 128) * 128))
    return caps


def pack_gather_bounce(lsi_core, caps):
    """lsi_core [T, BC, L] -> (g1x [128, T*CT/16] i16, g2x [128, T*512] i16)."""
    CT = sum(caps)
    CC = CT // 128
    lsi_core = np.asarray(lsi_core)
    g1w = []
    g2w = []
    for t in range(T):
        vals = lsi_core[t]                   # [BC, L]
        w_of = vals // W16
        loc = (vals - w_of * W16).astype(np.int16)
        g1 = np.zeros(CT, dtype=np.int16)
        g2 = np.zeros(L * BC, dtype=np.int16)
        off = 0
        for w in range(NW):
            sel = np.argwhere(w_of == w)     # [(s, l)] row-major
            n = len(sel)
            assert n <= caps[w], (t, w, n, caps[w])
            g1[off:off + n] = loc[sel[:, 0], sel[:, 1]]
            j = off + np.arange(n)
            rows = ((j % 128) * CC + j // 128).astype(np.int16)
            g2[sel[:, 1] * BC + sel[:, 0]] = rows
            off += caps[w]
        g1w.append(_wrap16(g1))
        g2w.append(_wrap16(g2))
    return (np.ascontiguousarray(np.concatenate(g1w, axis=1)),
            np.ascontiguousarray(np.concatenate(g2w, axis=1)))


_PROG = None
_PROG_KEY = None
TRACE = False
TRACE_KW = {}
LAST_RESULTS = None


def kernel(**inputs):
    global _PROG, _PROG_KEY, LAST_RESULTS
    dense_x = np.asarray(inputs["dense_x"], dtype=np.float32)     # [B, 13]
    lsi = np.asarray(inputs["lS_i"]).reshape(T, B, L)
    emb = np.asarray(inputs["emb"], dtype=np.float32)
    emb_flat = np.ascontiguousarray(emb.reshape(T * V, D))

    ws = {k: inputs[k] for k in (
        "Wb0", "bb0", "Wb1", "bb1", "Wb2", "bb2",
        "Wt0", "bt0", "Wt1", "bt1", "Wt2", "bt2")}
    packed_w = pack_weights(ws)

    caps = gather_caps(lsi) if GATHER_MODE == "bounce" else None
    key = (GATHER_MODE, tuple(caps) if caps else None)
    if _PROG is None or _PROG_KEY != key:
        _PROG = build_program(caps)
        _PROG_KEY = key
    nc = _PROG

    in_maps = []
    for c in range(NCORES):
        sl = slice(BC * c, BC * (c + 1))
        m = dict(packed_w)
        m["x_in"] = _bf16(dense_x[sl].T)
        m["emb"] = emb_flat
        if GATHER_MODE == "chain":
            m["gidx"] = pack_gidx_chain(lsi[:, sl, :])
        else:
            m["g1x"], m["g2x"] = pack_gather_bounce(lsi[:, sl, :], caps)
        in_maps.append(m)

    bkr = run_bass_kernel_spmd(
        nc, in_maps, list(range(NCORES)), trace=TRACE, **TRACE_KW)
    LAST_RESULTS = bkr
    outs = [np.asarray(r["out"]).reshape(BC) for r in bkr.results]
    return np.concatenate(outs).reshape(B, 1).astype(np.float32)
